# revision 4
# baseline (speedup 1.0000x reference)
# Bass/Tile kernel for nn_LstmAutoencoder on 8 Trainium2 NeuronCores — v2.
#
# Model: 128-step LSTM encoder (input 1, H=768) -> 128-step decoder (input
# constant 0 => bias-only input path) -> Linear(768->1) + softmax over a
# size-1 axis == 1.0 exactly, so the kernel's output stage is constant 1.0
# (same fold the baseline used).  The 256-step recurrence is computed
# faithfully: fp8 DoubleRow matmuls w/ fp32 PSUM, fp32 gate math.
#
# Key structure (per core, data-parallel batch 32):
#  - W matmuls keep hT stationary (batch cols), stream gate-major-permuted
#    weights; the 6 x 512-col output quads are partition-stacked in 3 PSUM
#    banks of [64, 512] (quad pair per bank at partition offsets 0/32), so
#    the fp32->bf16 convert is 3 ops of [64, 512] instead of 6 of [32, 512].
#  - convert -> PE re-transpose (regular matmuls vs identity) into
#    [128-partition, batch] tiles; a K=2 accumulate matmul fuses
#    bias + x_t * w_ih into the transposed tile (rows [ones; x_t]).
#  - gate math (sigmoid/tanh/cell update) runs at full 128 partitions, in
#    3 H-groups of 256 rows pipelined so each group's h is ready early for
#    the next step's matmuls (bank g consumes hT chunk g LAST).
#  - h stored directly as the fp8 DoubleRow stationary [128, 2, 32].
import functools
import sys

import numpy as np

sys.path.insert(0, "/opt/trn_rl_repo")

import ml_dtypes  # noqa: E402

import concourse.bass as bass  # noqa: E402
import concourse.mybir as mybir  # noqa: E402
from concourse import bacc  # noqa: E402
from concourse.bass_utils import run_bass_kernel_spmd  # noqa: E402
from concourse.masks import make_identity  # noqa: E402
from concourse.tile import TileContext  # noqa: E402

H = 768
G4 = 4 * H  # 3072
B = 256
NCORES = 8
BL = B // NCORES  # 32
KP = 3  # fp8 DoubleRow K chunk-pairs (256 rows each)
NQ = 6  # 512-col output quads
NOG = 3  # output groups (2 quads / 256 gate-rows each)
T_ENC = 128
T_DEC = 128

BF16 = mybir.dt.bfloat16
FP8 = mybir.dt.float8e4
F32 = mybir.dt.float32
AF = mybir.ActivationFunctionType
DR = mybir.MatmulPerfMode.DoubleRow

WSCALE = 16.0  # fp8 weight scale
HSCALE = 1.0  # fp8 hidden-state scale (unscaled; fp8-W error dominates)
PS_SCALE = 1.0 / (WSCALE * HSCALE)  # applied inside the sigmoid ACT op
# g-gate columns are host-prescaled x2 so tanh(x) = 2*sigmoid(2x)-1 comes out
# of the same sigmoid op; the 2*s-1 fixup is a fused DVE tensor_scalar.


def _gate_perm() -> np.ndarray:
    """Column permutation: col-tile j = 8*g + slot (g = output group) holds
    source gate rows gbase[slot] + 128*(2g + slot%2).  slot order per group:
    [i,i,f,f,o,o,g,g] so sigmoid = slots 0..5, tanh = slots 6..7."""
    gbase = [0, 0, H, H, 3 * H, 3 * H, 2 * H, 2 * H]  # i i f f o o g g
    idx = []
    for g in range(NOG):
        for slot in range(8):
            base = gbase[slot] + 128 * (2 * g + slot % 2)
            idx.extend(range(base, base + 128))
    return np.asarray(idx)


KP_ORDERS = {
    "rot0": ((0, 1, 2), (0, 1, 2), (0, 1, 2)),
    "rot1": ((1, 2, 0), (2, 0, 1), (0, 1, 2)),
    # early-stop: every bank starts on kp0 (first-ready), bank0 stops on the
    # early-ready kp1 so its convert/tail launches well before block end
    "estop": ((0, 2, 1), (0, 1, 2), (0, 1, 2)),
    "estop2": ((0, 2, 1), (0, 2, 1), (0, 1, 2)),
}


@functools.lru_cache(maxsize=8)
def _build(n_enc: int, n_dec: int, debug_out: bool,
           conv_eng: str = "avavav", kp_rot: str = "rot0", hp_off: int = 0):
    """conv_eng: per-OG convert engine, 'v'=DVE 'a'=ACT 'p'=Pool.
    kp_rot: per-bank kp-order preset from KP_ORDERS.
    hp_off: scheduler-priority boost for tail ops (0 disables)."""
    nc = bacc.Bacc(
        "TRN2", target_bir_lowering=False, debug=False, num_devices=NCORES
    )
    nsteps = n_enc + n_dec

    w_d = nc.dram_tensor("w", [128, 2 * KP * 2 * G4], FP8, kind="ExternalInput")
    bx_d = nc.dram_tensor("bx", [2, 2 * G4], BF16, kind="ExternalInput")
    xa_d = nc.dram_tensor(
        "xa", [2, max(1, nsteps) * BL], BF16, kind="ExternalInput"
    )
    out_d = nc.dram_tensor("out", [T_DEC, BL], F32, kind="ExternalOutput")
    if debug_out:
        hto_d = nc.dram_tensor("hT_out", [128, NOG * 2 * BL], BF16,
                               kind="ExternalOutput")
        cto_d = nc.dram_tensor("cT_out", [128, NOG * 2 * BL], F32,
                               kind="ExternalOutput")

    with TileContext(nc) as tc:
        with (
            tc.tile_pool(name="const", bufs=1) as cpool,
            tc.tile_pool(name="state", bufs=2) as spool,
            tc.tile_pool(name="work", bufs=2) as wpool,
            tc.tile_pool(name="gb", bufs=1, space="PSUM") as gbp,
            tc.tile_pool(name="pt", bufs=1, space="PSUM") as ptp,
        ):
            w_sb = cpool.tile_from(w_d[:, :])
            bx_sb = cpool.tile_from(bx_d[:, :])
            xa_sb = cpool.tile_from(xa_d[:, :])
            ident = cpool.tile([32, 32], BF16)
            make_identity(nc, ident)
            ones_sb = cpool.tile([BL, T_DEC], F32)
            nc.vector.memset(ones_sb, 1.0)

            def w_ap(ph, kp, q):
                """moving W for phase ph, chunk-pair kp, quad q: [128, 2, 512]."""
                blk = w_sb[
                    :, (ph * KP + kp) * 2 * G4 : (ph * KP + kp + 1) * 2 * G4
                ]
                return blk.rearrange("p (j n) -> p j n", j=2)[
                    :, :, 512 * q : 512 * (q + 1)
                ]

            hT = []
            cT = []
            for g in range(NOG):
                h = spool.tile([128, 2, BL], FP8, tag=f"h{g}", name=f"h{g}")
                nc.vector.memset(h, 0.0)
                hT.append(h)
                c = spool.tile([128, 2, BL], F32, tag=f"c{g}", name=f"c{g}")
                nc.vector.memset(c, 0.0)
                cT.append(c)

            conv_fns = {
                "v": lambda o, i: nc.vector.tensor_copy(o, i),
                "p": lambda o, i: nc.gpsimd.tensor_copy(o, i),
                "a": lambda o, i: nc.scalar.copy(o, i),
            }

            for t in range(nsteps):
                ph = 0 if t < n_enc else 1
                xsl = xa_sb[:, t * BL : (t + 1) * BL]
                gb = [
                    gbp.tile([32, 512], F32, tag=f"gb{q}", name=f"gb{q}")
                    for q in range(NQ)
                ]

                def mms_bank(g):
                    kps = KP_ORDERS[kp_rot][g]
                    for i, kp in enumerate(kps):
                        for q01 in range(2):
                            nc.tensor.matmul(
                                gb[2 * g + q01],
                                hT[kp],
                                w_ap(ph, kp, 2 * g + q01),
                                start=(i == 0),
                                stop=(i == 2),
                                perf_mode=DR,
                            )

                cnv = [None] * NQ

                def conv_bank(g):
                    for q in (2 * g, 2 * g + 1):
                        cv = wpool.tile([32, 512], BF16, tag=f"cnv{q}",
                                        name=f"cnv{q}")
                        conv_fns[conv_eng[q]](cv, gb[q])
                        cnv[q] = cv

                ptab = ptp.tile([128, 16, BL], F32, tag="ptA", name="ptA")
                ptb = ptp.tile([128, 8, BL], F32, tag="ptB", name="ptB")
                pre = [ptab[:, 0:8, :], ptab[:, 8:16, :], ptb]

                def txp_bank(g):
                    pt = pre[g]
                    for slot in range(8):
                        q = 2 * g + slot // 4
                        sub = slot % 4
                        nc.tensor.matmul(
                            pt[:, slot, :],
                            cnv[q][:, sub * 128 : (sub + 1) * 128],
                            ident,
                            start=True, stop=False,
                        )
                        j = 8 * g + slot
                        nc.tensor.matmul(
                            pt[:, slot, :],
                            bx_sb[:, ph * G4 + j * 128 : ph * G4 + (j + 1) * 128],
                            xsl,
                            start=False, stop=True,
                        )

                new_hT = [None] * NOG
                new_cT = [None] * NOG

                def sig_tg(g):
                    pt = pre[g]
                    # one sigmoid over all 4 gates ([i,i,f,f,o,o,g,g] slots;
                    # g cols pre-scaled x2 on host)
                    sig = wpool.tile([128, 8, BL], F32, tag=f"sig{g}",
                                     name=f"sig{g}")
                    nc.scalar.activation(sig, pt, AF.Sigmoid, scale=PS_SCALE)
                    # t1 = f*c on Pool (parallel w/ DVE); gg = 2*s-1 (= tanh),
                    # t2 = i*gg, cn = t1+t2 on DVE
                    t1 = wpool.tile([128, 2, BL], F32, tag=f"t1{g}",
                                    name=f"t1{g}")
                    nc.gpsimd.tensor_mul(t1, sig[:, 2:4, :], cT[g])
                    gg = wpool.tile([128, 2, BL], F32, tag=f"gg{g}",
                                    name=f"gg{g}")
                    nc.vector.tensor_scalar(
                        gg, sig[:, 6:8, :], 2.0, 1.0,
                        mybir.AluOpType.mult, mybir.AluOpType.subtract,
                    )
                    t2 = wpool.tile([128, 2, BL], F32, tag=f"t2{g}",
                                    name=f"t2{g}")
                    nc.vector.tensor_mul(t2, sig[:, 0:2, :], gg)
                    cn = spool.tile([128, 2, BL], F32, tag=f"c{g}",
                                    name=f"c{g}")
                    nc.vector.tensor_add(cn, t1, t2)
                    new_cT[g] = cn
                    return sig, cn

                sigs = [None] * NOG

                def tch_hb(g):
                    sig, cn = sigs[g]
                    tch = wpool.tile([128, 2, BL], F32, tag=f"tch{g}",
                                     name=f"tch{g}")
                    nc.scalar.activation(tch, cn, AF.Tanh)
                    # h tail-end on Pool: fast (full-rate TT) and idle, so no
                    # DVE-queue delay on the critical recurrence cycle.
                    # h stored unscaled fp8 (measured: state error identical
                    # with/without x16 — fp8-W quantization dominates).
                    hn = spool.tile([128, 2, BL], FP8, tag=f"h{g}",
                                    name=f"h{g}")
                    nc.gpsimd.tensor_mul(hn, sig[:, 4:6, :], tch)
                    new_hT[g] = hn

                # emission schedule: stagger per-bank tails; tail ops get
                # elevated scheduler priority so the PE interrupts the W
                # stream to launch each group's transpose as soon as its
                # convert lands.
                mms_bank(0)
                mms_bank(1)
                with tc.high_priority(offset=hp_off):
                    conv_bank(0)
                    txp_bank(0)
                mms_bank(2)
                with tc.high_priority(offset=hp_off):
                    conv_bank(1)
                    sigs[0] = sig_tg(0)
                    txp_bank(1)
                    conv_bank(2)
                    sigs[1] = sig_tg(1)
                    tch_hb(0)
                    txp_bank(2)
                    sigs[2] = sig_tg(2)
                    tch_hb(1)
                    tch_hb(2)
                hT = new_hT
                cT = new_cT

            nc.sync.dma_start(out=out_d[:, :].rearrange("t b -> b t"),
                              in_=ones_sb)
            if debug_out:
                for g in range(NOG):
                    hd = wpool.tile([128, 2, BL], BF16, tag="hdbg",
                                    name="hdbg")
                    nc.vector.tensor_scalar_mul(hd, hT[g], 1.0 / HSCALE)
                    nc.sync.dma_start(
                        out=hto_d[:, g * 2 * BL : (g + 1) * 2 * BL]
                        .rearrange("p (j b) -> p j b", j=2),
                        in_=hd,
                    )
                    nc.sync.dma_start(
                        out=cto_d[:, g * 2 * BL : (g + 1) * 2 * BL]
                        .rearrange("p (j b) -> p j b", j=2),
                        in_=cT[g],
                    )
    nc.compile()
    return nc


def _col_scale():
    """Extra x2 on g-gate column tiles (slots 6,7 of each group of 8)."""
    s = np.ones(G4, np.float32)
    for g in range(NOG):
        s[(8 * g + 6) * 128 : (8 * g + 8) * 128] = 2.0
    return s


def _prep_inputs(inputs):
    perm = _gate_perm()
    bf = ml_dtypes.bfloat16
    f8 = ml_dtypes.float8_e4m3
    cs = _col_scale()

    def wprep(w_hh):
        rhs = np.ascontiguousarray(np.asarray(w_hh, np.float32)[perm, :].T)
        rhs = rhs * cs[None, :]
        arr = rhs.reshape(KP, 2, 128, G4).transpose(2, 0, 1, 3)
        return (arr * WSCALE).reshape(128, KP * 2 * G4).astype(f8)

    wenc = wprep(inputs["w_hh_enc"])
    wdec = wprep(inputs["w_hh_dec"])
    w = np.concatenate([wenc, wdec], axis=1)  # [128, 2*KP*2*G4]

    bsc = WSCALE * HSCALE * cs
    benc = (np.asarray(inputs["b_ih_enc"], np.float32)
            + np.asarray(inputs["b_hh_enc"], np.float32))[perm] * bsc
    bdec = (np.asarray(inputs["b_ih_dec"], np.float32)
            + np.asarray(inputs["b_hh_dec"], np.float32))[perm] * bsc
    wie = np.asarray(inputs["w_ih_enc"], np.float32)[perm, 0] * bsc
    bx = np.stack([
        np.concatenate([benc, bdec]),
        np.concatenate([wie, np.zeros(G4, np.float32)]),
    ]).astype(bf)  # [2, 2*G4]
    return w, bx


def _make_inmaps(inputs, n_enc: int, n_dec: int):
    w, bx = _prep_inputs(inputs)
    nsteps = n_enc + n_dec
    x = np.asarray(inputs["x"], np.float32)  # [T, B, 1]
    bf = ml_dtypes.bfloat16
    in_maps = []
    for c in range(NCORES):
        xa = np.zeros((2, max(1, nsteps) * BL), np.float32)
        xa[0, :] = 1.0
        xloc = x[:n_enc, c * BL : (c + 1) * BL, 0]
        xa[1, : n_enc * BL] = xloc.reshape(-1)
        in_maps.append({"w": w, "bx": bx, "xa": xa.astype(bf)})
    return in_maps


def run_steps(inputs, n_enc: int, n_dec: int, debug_out: bool = False,
              trace: bool = False, conv_eng: str = "avavav", kp_rot: str = "rot0"):
    nc = _build(n_enc, n_dec, debug_out, conv_eng, kp_rot)
    in_maps = _make_inmaps(inputs, n_enc, n_dec)
    res = run_bass_kernel_spmd(nc, in_maps, list(range(NCORES)), trace=trace)
    return res.results, res


def kernel(**inputs) -> np.ndarray:
    results, _ = run_steps(inputs, T_ENC, T_DEC, debug_out=False)
    out = np.empty((T_DEC, B, 1), np.float32)
    for c in range(NCORES):
        out[:, c * BL : (c + 1) * BL, 0] = results[c]["out"]
    return out
